# revision 17
# baseline (speedup 1.0000x reference)
"""Trainium2 Bass kernel for nn_DiffusionHead (32-step DDPM sampling head).

Strategy (pure data parallel, 8 cores, B=65536 -> 8192 rows/core):
  - Transposed activation layout: features on SBUF partitions, batch rows on
    the free dimension. All matmuls in float32r (fp32 storage, ~tf32
    precision, full PE rate).
  - The time-embedding contribution of layer 1 is folded into a per-step
    bias table on the host. The cond contribution of layer 1 (cond @ W1c)
    is step-invariant: computed once on device, kept resident in SBUF.
  - The DDPM update  x_{i+1} = A_i x_i + B_i eps_i + C_i n_i + B_i b4  is
    evaluated inside the layer-4 PSUM accumulation:
      * B_i is folded into a per-step copy of W4 (host table),
      * A_i x_i and the pre-scaled noise are added via one extra matmul
        with a host-built [128,64] block-diagonal operand against a
        combined state tile X = [x^T ; noiseC''^T],
      * C_i n_i + B_i b4 is pre-computed on the host into the noise stream.
    One DVE copy per block moves PSUM into the next state tile.
"""

import math
import os
import sys
import types

import numpy as np

sys.path.insert(0, "/opt/trn_rl_repo")

# ---------------------------------------------------------------- constants
T = 32
BETA_START, BETA_END = 1e-4, 1e-2
ACTION_DIM, ACTION_HORIZON = 4, 16
CHUNK = ACTION_DIM * ACTION_HORIZON          # 64
COND_DIM = 256
TIME_EMB_DIM = 32
HIDDEN = 256
B = 65536
N_CORES = 8
R = B // N_CORES                             # 8192 rows per core
FD = 1024                                    # elementwise block free-dim
NB = R // FD                                 # 8 blocks per step
FDH = 512                                    # matmul moving free-dim (psum bank)


def _host_tables(W1, b1, W4, b4):
    """Per-step coefficient tables, computed in float64 then cast."""
    betas = np.linspace(BETA_START, BETA_END, T, dtype=np.float64)
    alphas = 1.0 - betas
    alpha_bar = np.cumprod(alphas)
    freqs = np.exp(np.linspace(0.0, math.log(1000.0), TIME_EMB_DIM // 2))

    A = np.empty(T)
    Bc = np.empty(T)
    Cc = np.empty(T)
    for i in range(T):
        t = T - 1 - i
        ab, al, be = alpha_bar[t], alphas[t], betas[t]
        if t > 0:
            A[i] = math.sqrt(al / ab)
            Bc[i] = -math.sqrt(al * (1.0 - ab) / ab)
            Cc[i] = math.sqrt(be)
        else:
            A[i] = 1.0 / math.sqrt(ab)
            Bc[i] = -math.sqrt((1.0 - ab) / ab)
            Cc[i] = 0.0

    # layer-1 bias per step: b1 + t_emb(t) @ W1[64:96]
    W1 = W1.astype(np.float64)
    b1t = np.empty((T, HIDDEN))
    for i in range(T):
        t = T - 1 - i
        args = t * freqs
        temb = np.concatenate([np.sin(args), np.cos(args)])
        b1t[i] = b1.astype(np.float64) + temb @ W1[CHUNK:CHUNK + TIME_EMB_DIM]

    # b1t table for SBUF: [128, T*2], [p, i*2+m] = b1t[i][m*128+p]
    b1t_d = np.empty((128, T * 2), dtype=np.float32)
    for i in range(T):
        for m in range(2):
            b1t_d[:, i * 2 + m] = b1t[i, m * 128:(m + 1) * 128]

    # W4 table: [2, 128, T*64], [k, p, i*64+m] = B_i * W4[k*128+p, m]
    W4_d = np.empty((2, 128, T * 64), dtype=np.float32)
    for i in range(T):
        for k in range(2):
            W4_d[k][:, i * 64:(i + 1) * 64] = Bc[i] * W4[k * 128:(k + 1) * 128, :]

    # state/noise matmul operand: [128, T*64]
    # rows 0..63:  A_i * I64   (x part);  rows 64..127: I64 (noise part)
    L2_d = np.zeros((128, T * 64), dtype=np.float32)
    eye = np.eye(CHUNK, dtype=np.float32)
    for i in range(T):
        L2_d[0:64, i * 64:(i + 1) * 64] = A[i] * eye
        L2_d[64:128, i * 64:(i + 1) * 64] = eye

    return A, Bc, Cc, b1t_d, W4_d, L2_d


def _gen_noise(b4, Bc, Cc):
    """Reference-matching threefry noise, pre-scaled:
    noiseC[i] = C_i * n_i^T + B_i * b4[:, None]   shape [T, 64, B]"""
    import jax
    import jax.numpy as jnp

    out = np.empty((T, CHUNK, B), dtype=np.float32)
    keys = jax.random.split(jax.random.key(42), T)
    for i in range(T):
        if Cc[i] != 0.0:
            n = np.asarray(jax.random.normal(keys[i], (B, CHUNK), jnp.float32))
            out[i] = (np.float32(Cc[i]) * n.T
                      + np.float32(Bc[i]) * b4[:, None])
        else:
            out[i] = (np.float32(Bc[i]) * b4[:, None]) * np.ones((1, B), np.float32)
    return out



def _enable_ldw_opt():
    """Flip walrus --enable-ldw-opt to true (dedups repeated weight loads;
    halves PE LDWEIGHTS time for our consecutive same-weight matmuls)."""
    return  # disabled: conflicts with explicit bf16 InstLdweights
    from concourse import bass_utils as _bu
    if getattr(_bu, "_ldw_opt_patched", False):
        return
    _orig = _bu.run_command

    def _run(cmd, *a, **kw):
        if isinstance(cmd, list) and "--enable-ldw-opt=false" in cmd:
            cmd = ["--enable-ldw-opt=true" if c == "--enable-ldw-opt=false" else c
                   for c in cmd]
        return _orig(cmd, *a, **kw)

    _bu.run_command = _run
    _bu._ldw_opt_patched = True

def _build_nc():
    import concourse.bacc as bacc
    import concourse.mybir as mybir
    from concourse.tile import TileContext

    F32R = mybir.dt.float32r
    F32 = mybir.dt.float32
    SILU = mybir.ActivationFunctionType.Silu
    ADD = mybir.AluOpType.add

    nc = bacc.Bacc("TRN2", target_bir_lowering=False, debug=False,
                   num_devices=N_CORES)

    xT = nc.dram_tensor("xT", [CHUNK, R], F32, kind="ExternalInput").ap()
    condT = nc.dram_tensor("condT", [COND_DIM, R], F32R, kind="ExternalInput").ap()
    noise = nc.dram_tensor("noise", [T, CHUNK, R], F32R, kind="ExternalInput").ap()
    W1a_d = nc.dram_tensor("W1a", [128, 128], mybir.dt.bfloat16, kind="ExternalInput").ap()
    W1c_d = nc.dram_tensor("W1c", [2, 128, HIDDEN], F32R, kind="ExternalInput").ap()
    BF16D = mybir.dt.bfloat16
    W2_d = nc.dram_tensor("W2d", [2, 128, HIDDEN], BF16D, kind="ExternalInput").ap()
    W3_d = nc.dram_tensor("W3d", [2, 128, HIDDEN], BF16D, kind="ExternalInput").ap()
    W4_d = nc.dram_tensor("W4d", [2, 128, T * 64], BF16D, kind="ExternalInput").ap()
    L2_d = nc.dram_tensor("L2d", [128, T * 64], F32R, kind="ExternalInput").ap()
    b1t_d = nc.dram_tensor("b1td", [128, T * 2], F32, kind="ExternalInput").ap()
    b2_d = nc.dram_tensor("b2d", [128, 2], F32, kind="ExternalInput").ap()
    b3_d = nc.dram_tensor("b3d", [128, 2], F32, kind="ExternalInput").ap()
    outT = nc.dram_tensor("outT", [CHUNK, R], F32R, kind="ExternalOutput").ap()

    with TileContext(nc) as tc:
        with tc.tile_pool(name="persist", bufs=1) as pp, \
             tc.tile_pool(name="work", bufs=2) as wp, \
             tc.tile_pool(name="psum", bufs=3, space="PSUM") as psp, \
             tc.tile_pool(name="psum4", bufs=1, space="PSUM") as ps4p:

            # ---------------- persistent tiles
            X = [pp.tile([128, R], F32R, tag="x0", name="x0"),
                 pp.tile([128, R], F32R, tag="x1", name="x1")]
            BF16 = mybir.dt.bfloat16
            pT = [pp.tile([128, R], BF16, tag="pt0", name="pt0"),
                  pp.tile([128, R], BF16, tag="pt1", name="pt1")]
            W1a = pp.tile([128, 128], mybir.dt.bfloat16, tag="w1a")
            xd = pp.tile([128, R], mybir.dt.bfloat16, tag="xd", name="xd")
            W2 = pp.tile([128, 2 * HIDDEN], mybir.dt.bfloat16, tag="w2")
            W3 = pp.tile([128, 2 * HIDDEN], mybir.dt.bfloat16, tag="w3")
            W4 = pp.tile([128, 2 * T * 64], mybir.dt.bfloat16, tag="w4")
            L2s = pp.tile([128, T * 64], F32R, tag="l2s")
            b1t = pp.tile([128, T * 2], F32, tag="b1t")
            b2 = pp.tile([128, 2], F32, tag="b2")
            b3 = pp.tile([128, 2], F32, tag="b3")

            nc.sync.dma_start(out=W1a[:], in_=W1a_d)
            for k in range(2):
                nc.sync.dma_start(out=W2[:, k * 256:(k + 1) * 256], in_=W2_d[k])
                nc.sync.dma_start(out=W3[:, k * 256:(k + 1) * 256], in_=W3_d[k])
                nc.sync.dma_start(out=W4[:, k * T * 64:(k + 1) * T * 64], in_=W4_d[k])
            nc.sync.dma_start(out=L2s[:], in_=L2_d)
            nc.sync.dma_start(out=b1t[:], in_=b1t_d)
            nc.sync.dma_start(out=b2[:], in_=b2_d)
            nc.sync.dma_start(out=b3[:], in_=b3_d)
            nc.gpsimd.dma_start(out=X[0][0:CHUNK, :], in_=xT)
            nc.gpsimd.dma_start(out=xd[0:CHUNK, :], in_=xT)
            nc.gpsimd.dma_start(out=xd[CHUNK:128, :], in_=xT)
            nc.sync.dma_start(out=X[0][CHUNK:128, :], in_=noise[0])

            # ---------------- one-time cond projection: pT = (cond @ W1c)^T
            with tc.tile_pool(name="init", bufs=2) as ip:
                W1c = ip.tile([128, 2 * HIDDEN], F32R, tag="w1c", bufs=1)
                for k in range(2):
                    nc.sync.dma_start(out=W1c[:, k * 256:(k + 1) * 256], in_=W1c_d[k])
                for blk in range(NB):
                    r0 = blk * FD
                    ct = [[None, None], [None, None]]
                    for k in range(2):
                        for n in range(2):
                            c = ip.tile([128, FDH], F32R, tag=f"c{k}{n}",
                                        name=f"ct{k}{n}", bufs=1)
                            nc.sync.dma_start(
                                out=c[:],
                                in_=condT[k * 128:(k + 1) * 128,
                                          r0 + n * FDH: r0 + (n + 1) * FDH])
                            ct[k][n] = c
                    for m in range(2):
                        ps = psp.tile([128, FD], F32, tag="ps")
                        for k in range(2):
                            for n in range(2):
                                nc.tensor.matmul(
                                    ps[:, n * FDH:(n + 1) * FDH],
                                    W1c[:, k * 256 + m * 128: k * 256 + (m + 1) * 128],
                                    ct[k][n][:],
                                    start=(k == 0), stop=(k == 1))
                        nc.vector.tensor_copy(pT[m][:, r0:r0 + FD], ps[:])

            # ---------------- 32 denoising steps (phase-major in waves of 4
            # blocks: deeper PE lookahead within each layer phase)
            BF16 = mybir.dt.bfloat16
            WAVE = 4
            for i in range(T):
                Xc = X[i % 2]
                Xn = X[(i + 1) % 2]
                if i + 1 < T:
                    nc.sync.dma_start(out=Xn[CHUNK:128, :], in_=noise[i + 1])
                for w in range(NB // WAVE):
                    blocks = [w * WAVE + j for j in range(WAVE)]

                    # L1 + cond add + silu
                    h1 = {}
                    for blk in blocks:
                        r0 = blk * FD
                        psm = [psp.tile([128, FD], F32, tag="ps", name="psm")
                               for _ in range(2)]
                        for n in range(2):
                            for m in range(2):
                                nc.tensor.matmul(
                                    psm[m][:, n * FDH:(n + 1) * FDH],
                                    W1a[m * CHUNK:(m + 1) * CHUNK, :],
                                    xd[m * CHUNK:(m + 1) * CHUNK,
                                       r0 + n * FDH: r0 + (n + 1) * FDH],
                                    start=True, stop=True)
                        for m in range(2):
                            ps = psm[m]
                            a1 = wp.tile([128, FD], F32, tag=f"a1_{m}",
                                         name=f"a1_{m}", bufs=2)
                            nc.vector.tensor_tensor(a1[:], ps[:],
                                                    pT[m][:, r0:r0 + FD], op=ADD)
                            h = wp.tile([128, FD], BF16, tag=f"ha_{m}", bufs=6)
                            nc.scalar.activation(h[:], a1[:], SILU,
                                                 bias=b1t[:, i * 2 + m: i * 2 + m + 1])
                            h1[(blk, m)] = h

                    # L2
                    h2 = {}
                    for blk in blocks:
                        for m in range(2):
                            ps = psp.tile([128, FD], F32, tag="ps")
                            for k in range(2):
                                for n in range(2):
                                    nc.tensor.matmul(
                                        ps[:, n * FDH:(n + 1) * FDH],
                                        W2[:, k * 256 + m * 128: k * 256 + (m + 1) * 128],
                                        h1[(blk, k)][:, n * FDH:(n + 1) * FDH],
                                        start=(k == 0), stop=(k == 1))
                            h = wp.tile([128, FD], BF16, tag=f"h2_{m}", bufs=6)
                            nc.scalar.activation(h[:], ps[:], SILU, bias=b2[:, m:m + 1])
                            h2[(blk, m)] = h

                    # L3
                    h3 = {}
                    for blk in blocks:
                        for m in range(2):
                            ps = psp.tile([128, FD], F32, tag="ps")
                            for k in range(2):
                                for n in range(2):
                                    nc.tensor.matmul(
                                        ps[:, n * FDH:(n + 1) * FDH],
                                        W3[:, k * 256 + m * 128: k * 256 + (m + 1) * 128],
                                        h2[(blk, k)][:, n * FDH:(n + 1) * FDH],
                                        start=(k == 0), stop=(k == 1))
                            h = wp.tile([128, FD], BF16, tag=f"ha_{m}", bufs=6)
                            nc.scalar.activation(h[:], ps[:], SILU, bias=b3[:, m:m + 1])
                            h3[(blk, m)] = h

                    # L4 + state/noise add, then copy psum -> next state
                    for blk in blocks:
                        r0 = blk * FD
                        p4 = [ps4p.tile([CHUNK, FDH], F32, tag="ps4",
                                        name="p4", bufs=2) for _ in range(2)]
                        for k in range(2):
                            for n in range(2):
                                nc.tensor.matmul(
                                    p4[n][:],
                                    W4[:, k * T * 64 + i * 64: k * T * 64 + (i + 1) * 64],
                                    h3[(blk, k)][:, n * FDH:(n + 1) * FDH],
                                    start=(k == 0), stop=False)
                        for n in range(2):
                            rs = slice(r0 + n * FDH, r0 + (n + 1) * FDH)
                            nc.tensor.matmul(
                                p4[n][:],
                                L2s[:, i * 64:(i + 1) * 64], Xc[:, rs],
                                start=False, stop=True)
                            nc.vector.tensor_copy(Xn[0:CHUNK, rs], p4[n][:])
                        if i + 1 < T:
                            nc.vector.tensor_copy(xd[0:CHUNK, r0:r0 + FD],
                                                  Xn[0:CHUNK, r0:r0 + FD])
                            nc.vector.tensor_copy(xd[CHUNK:128, r0:r0 + FD],
                                                  Xn[0:CHUNK, r0:r0 + FD])

            nc.sync.dma_start(out=outT, in_=X[T % 2][0:CHUNK, :])

    nc.finalize()
    return nc


def _prep_inputs(cond, x_init, W1, b1, W2, b2, W3, b3, W4, b4):
    A, Bc, Cc, b1t_d, W4_d, L2_d = _host_tables(W1, b1, W4, b4)
    noiseC = _gen_noise(b4.astype(np.float32), Bc, Cc)

    import ml_dtypes
    W1af = W1[:CHUNK].astype(np.float32)
    W1a = np.empty((128, 128), dtype=ml_dtypes.bfloat16)
    W1a[0:CHUNK] = W1af[:, 0:128]
    W1a[CHUNK:128] = W1af[:, 128:256]
    W1c = np.ascontiguousarray(
        W1[CHUNK + TIME_EMB_DIM:].reshape(2, 128, HIDDEN)).astype(np.float32)
    W2r = np.ascontiguousarray(W2.reshape(2, 128, HIDDEN)).astype(ml_dtypes.bfloat16)
    W3r = np.ascontiguousarray(W3.reshape(2, 128, HIDDEN)).astype(ml_dtypes.bfloat16)

    def btab(b):
        o = np.empty((128, 2), dtype=np.float32)
        for m in range(2):
            o[:, m] = b[m * 128:(m + 1) * 128]
        return o

    b2_d, b3_d = btab(b2.astype(np.float32)), btab(b3.astype(np.float32))

    condT = np.ascontiguousarray(cond.T).astype(np.float32)      # [256, B]
    xT = np.ascontiguousarray(x_init.T).astype(np.float32)       # [64, B]

    in_maps = []
    for c in range(N_CORES):
        rc = slice(c * R, (c + 1) * R)
        in_maps.append({
            "xT": np.ascontiguousarray(xT[:, rc]),
            "condT": np.ascontiguousarray(condT[:, rc]),
            "noise": np.ascontiguousarray(noiseC[:, :, rc]),
            "W1a": W1a, "W1c": W1c, "W2d": W2r, "W3d": W3r,
            "W4d": W4_d.astype(ml_dtypes.bfloat16), "L2d": L2_d, "b1td": b1t_d,
            "b2d": b2_d, "b3d": b3_d,
        })
    return in_maps


def _install_profile_hook():
    """The agent image's antenv lacks axon_hooks; recreate it so trace=True
    works (used by test.py only)."""
    try:
        import antenv
        if "antenv.axon_hooks" in sys.modules:
            return
        mod = types.ModuleType("antenv.axon_hooks")
        hook = [None]
        mod.set_axon_ntff_profile_hook = lambda h: hook.__setitem__(0, h)
        mod.get_axon_ntff_profile_hook = lambda: hook[0]
        sys.modules["antenv.axon_hooks"] = mod
        antenv.axon_hooks = mod
        from trn_agent_boot.trn_boot import _ntff_profile_via_ctypes
        mod.set_axon_ntff_profile_hook(
            _ntff_profile_via_ctypes("/opt/axon/libaxon_pjrt.so"))
    except Exception:
        pass


_LAST_RESULTS = {}


def kernel(cond, x_init, W1, b1, W2, b2, W3, b3, W4, b4, _trace=False):
    _install_profile_hook()
    from concourse import bass_utils
    bass_utils.upload_artifacts = lambda tmpdir: "local://" + tmpdir
    from concourse.bass_utils import run_bass_kernel_spmd

    _enable_ldw_opt()
    in_maps = _prep_inputs(cond, x_init, W1, b1, W2, b2, W3, b3, W4, b4)
    nc = _build_nc()
    res = run_bass_kernel_spmd(nc, in_maps, list(range(N_CORES)), trace=_trace)
    _LAST_RESULTS["res"] = res

    out = np.empty((B, CHUNK), dtype=np.float32)
    for c in range(N_CORES):
        rc = slice(c * R, (c + 1) * R)
        out[rc] = res.results[c]["outT"].T
    return out.reshape(B, ACTION_HORIZON, ACTION_DIM)


# revision 18
# speedup vs baseline: 1.1329x; 1.1329x over previous
"""Trainium2 Bass kernel for nn_DiffusionHead (32-step DDPM sampling head).

Strategy (pure data parallel, 8 cores, B=65536 -> 8192 rows/core):
  - Transposed activation layout: features on SBUF partitions, batch rows on
    the free dimension. All matmuls in float32r (fp32 storage, ~tf32
    precision, full PE rate).
  - The time-embedding contribution of layer 1 is folded into a per-step
    bias table on the host. The cond contribution of layer 1 (cond @ W1c)
    is step-invariant: computed once on device, kept resident in SBUF.
  - The DDPM update  x_{i+1} = A_i x_i + B_i eps_i + C_i n_i + B_i b4  is
    evaluated inside the layer-4 PSUM accumulation:
      * B_i is folded into a per-step copy of W4 (host table),
      * A_i x_i and the pre-scaled noise are added via one extra matmul
        with a host-built [128,64] block-diagonal operand against a
        combined state tile X = [x^T ; noiseC''^T],
      * C_i n_i + B_i b4 is pre-computed on the host into the noise stream.
    One DVE copy per block moves PSUM into the next state tile.
"""

import math
import os
import sys
import types

import numpy as np

sys.path.insert(0, "/opt/trn_rl_repo")

# ---------------------------------------------------------------- constants
T = 32
BETA_START, BETA_END = 1e-4, 1e-2
ACTION_DIM, ACTION_HORIZON = 4, 16
CHUNK = ACTION_DIM * ACTION_HORIZON          # 64
COND_DIM = 256
TIME_EMB_DIM = 32
HIDDEN = 256
B = 65536
N_CORES = 8
R = B // N_CORES                             # 8192 rows per core
FD = 1024                                    # elementwise block free-dim
NB = R // FD                                 # 8 blocks per step
FDH = 512                                    # matmul moving free-dim (psum bank)


def _host_tables(W1, b1, W4, b4):
    """Per-step coefficient tables, computed in float64 then cast."""
    betas = np.linspace(BETA_START, BETA_END, T, dtype=np.float64)
    alphas = 1.0 - betas
    alpha_bar = np.cumprod(alphas)
    freqs = np.exp(np.linspace(0.0, math.log(1000.0), TIME_EMB_DIM // 2))

    A = np.empty(T)
    Bc = np.empty(T)
    Cc = np.empty(T)
    for i in range(T):
        t = T - 1 - i
        ab, al, be = alpha_bar[t], alphas[t], betas[t]
        if t > 0:
            A[i] = math.sqrt(al / ab)
            Bc[i] = -math.sqrt(al * (1.0 - ab) / ab)
            Cc[i] = math.sqrt(be)
        else:
            A[i] = 1.0 / math.sqrt(ab)
            Bc[i] = -math.sqrt((1.0 - ab) / ab)
            Cc[i] = 0.0

    # layer-1 bias per step: b1 + t_emb(t) @ W1[64:96]
    W1 = W1.astype(np.float64)
    b1t = np.empty((T, HIDDEN))
    for i in range(T):
        t = T - 1 - i
        args = t * freqs
        temb = np.concatenate([np.sin(args), np.cos(args)])
        b1t[i] = b1.astype(np.float64) + temb @ W1[CHUNK:CHUNK + TIME_EMB_DIM]

    # b1t table for SBUF: [128, T*2], [p, i*2+m] = b1t[i][m*128+p]
    b1t_d = np.empty((128, T * 2), dtype=np.float32)
    for i in range(T):
        for m in range(2):
            b1t_d[:, i * 2 + m] = b1t[i, m * 128:(m + 1) * 128]

    # W4 table: [2, 128, T*64], [k, p, i*64+m] = B_i * W4[k*128+p, m]
    W4_d = np.empty((2, 128, T * 64), dtype=np.float32)
    for i in range(T):
        for k in range(2):
            W4_d[k][:, i * 64:(i + 1) * 64] = Bc[i] * W4[k * 128:(k + 1) * 128, :]

    # state/noise matmul operand: [128, T*64]
    # rows 0..63:  A_i * I64   (x part);  rows 64..127: I64 (noise part)
    L2_d = np.zeros((128, T * 64), dtype=np.float32)
    eye = np.eye(CHUNK, dtype=np.float32)
    for i in range(T):
        L2_d[0:64, i * 64:(i + 1) * 64] = A[i] * eye
        L2_d[64:128, i * 64:(i + 1) * 64] = eye

    return A, Bc, Cc, b1t_d, W4_d, L2_d


def _gen_noise(b4, Bc, Cc):
    """Reference-matching threefry noise, pre-scaled:
    noiseC[i] = C_i * n_i^T + B_i * b4[:, None]   shape [T, 64, B]"""
    import jax
    import jax.numpy as jnp

    out = np.empty((T, CHUNK, B), dtype=np.float32)
    keys = jax.random.split(jax.random.key(42), T)
    for i in range(T):
        if Cc[i] != 0.0:
            n = np.asarray(jax.random.normal(keys[i], (B, CHUNK), jnp.float32))
            out[i] = (np.float32(Cc[i]) * n.T
                      + np.float32(Bc[i]) * b4[:, None])
        else:
            out[i] = (np.float32(Bc[i]) * b4[:, None]) * np.ones((1, B), np.float32)
    return out



def _enable_ldw_opt():
    """Flip walrus --enable-ldw-opt to true (dedups repeated weight loads;
    halves PE LDWEIGHTS time for our consecutive same-weight matmuls)."""
    return  # disabled: conflicts with explicit bf16 InstLdweights
    from concourse import bass_utils as _bu
    if getattr(_bu, "_ldw_opt_patched", False):
        return
    _orig = _bu.run_command

    def _run(cmd, *a, **kw):
        if isinstance(cmd, list) and "--enable-ldw-opt=false" in cmd:
            cmd = ["--enable-ldw-opt=true" if c == "--enable-ldw-opt=false" else c
                   for c in cmd]
        return _orig(cmd, *a, **kw)

    _bu.run_command = _run
    _bu._ldw_opt_patched = True

def _build_nc():
    import concourse.bacc as bacc
    import concourse.mybir as mybir
    from concourse.tile import TileContext

    F32R = mybir.dt.float32r
    F32 = mybir.dt.float32
    SILU = mybir.ActivationFunctionType.Silu
    ADD = mybir.AluOpType.add

    nc = bacc.Bacc("TRN2", target_bir_lowering=False, debug=False,
                   num_devices=N_CORES)

    xT = nc.dram_tensor("xT", [CHUNK, R], F32R, kind="ExternalInput").ap()
    condT = nc.dram_tensor("condT", [COND_DIM, R], F32R, kind="ExternalInput").ap()
    noise = nc.dram_tensor("noise", [T, CHUNK, R], F32R, kind="ExternalInput").ap()
    W1a_d = nc.dram_tensor("W1a", [CHUNK, HIDDEN], F32R, kind="ExternalInput").ap()
    W1c_d = nc.dram_tensor("W1c", [2, 128, HIDDEN], F32R, kind="ExternalInput").ap()
    BF16D = mybir.dt.bfloat16
    W2_d = nc.dram_tensor("W2d", [2, 128, HIDDEN], BF16D, kind="ExternalInput").ap()
    W3_d = nc.dram_tensor("W3d", [2, 128, HIDDEN], BF16D, kind="ExternalInput").ap()
    W4_d = nc.dram_tensor("W4d", [2, 128, T * 64], BF16D, kind="ExternalInput").ap()
    L2_d = nc.dram_tensor("L2d", [128, T * 64], F32R, kind="ExternalInput").ap()
    b1t_d = nc.dram_tensor("b1td", [128, T * 2], F32, kind="ExternalInput").ap()
    b2_d = nc.dram_tensor("b2d", [128, 2], F32, kind="ExternalInput").ap()
    b3_d = nc.dram_tensor("b3d", [128, 2], F32, kind="ExternalInput").ap()
    outT = nc.dram_tensor("outT", [CHUNK, R], F32R, kind="ExternalOutput").ap()

    with TileContext(nc) as tc:
        with tc.tile_pool(name="persist", bufs=1) as pp, \
             tc.tile_pool(name="work", bufs=2) as wp, \
             tc.tile_pool(name="psum", bufs=3, space="PSUM") as psp, \
             tc.tile_pool(name="psum4", bufs=1, space="PSUM") as ps4p:

            # ---------------- persistent tiles
            X = [pp.tile([128, R], F32R, tag="x0", name="x0"),
                 pp.tile([128, R], F32R, tag="x1", name="x1")]
            BF16 = mybir.dt.bfloat16
            pT = [pp.tile([128, R], BF16, tag="pt0", name="pt0"),
                  pp.tile([128, R], BF16, tag="pt1", name="pt1")]
            W1a = pp.tile([CHUNK, HIDDEN], F32R, tag="w1a")
            W2 = pp.tile([128, 2 * HIDDEN], mybir.dt.bfloat16, tag="w2")
            W3 = pp.tile([128, 2 * HIDDEN], mybir.dt.bfloat16, tag="w3")
            W4 = pp.tile([128, 2 * T * 64], mybir.dt.bfloat16, tag="w4")
            L2s = pp.tile([128, T * 64], F32R, tag="l2s")
            b1t = pp.tile([128, T * 2], F32, tag="b1t")
            b2 = pp.tile([128, 2], F32, tag="b2")
            b3 = pp.tile([128, 2], F32, tag="b3")

            nc.sync.dma_start(out=W1a[:], in_=W1a_d)
            for k in range(2):
                nc.sync.dma_start(out=W2[:, k * 256:(k + 1) * 256], in_=W2_d[k])
                nc.sync.dma_start(out=W3[:, k * 256:(k + 1) * 256], in_=W3_d[k])
                nc.sync.dma_start(out=W4[:, k * T * 64:(k + 1) * T * 64], in_=W4_d[k])
            nc.sync.dma_start(out=L2s[:], in_=L2_d)
            nc.sync.dma_start(out=b1t[:], in_=b1t_d)
            nc.sync.dma_start(out=b2[:], in_=b2_d)
            nc.sync.dma_start(out=b3[:], in_=b3_d)
            nc.sync.dma_start(out=X[0][0:CHUNK, :], in_=xT)
            nc.sync.dma_start(out=X[0][CHUNK:128, :], in_=noise[0])

            # ---------------- one-time cond projection: pT = (cond @ W1c)^T
            with tc.tile_pool(name="init", bufs=2) as ip:
                W1c = ip.tile([128, 2 * HIDDEN], F32R, tag="w1c", bufs=1)
                for k in range(2):
                    nc.sync.dma_start(out=W1c[:, k * 256:(k + 1) * 256], in_=W1c_d[k])
                for blk in range(NB):
                    r0 = blk * FD
                    ct = [[None, None], [None, None]]
                    for k in range(2):
                        for n in range(2):
                            c = ip.tile([128, FDH], F32R, tag=f"c{k}{n}",
                                        name=f"ct{k}{n}", bufs=1)
                            nc.sync.dma_start(
                                out=c[:],
                                in_=condT[k * 128:(k + 1) * 128,
                                          r0 + n * FDH: r0 + (n + 1) * FDH])
                            ct[k][n] = c
                    for m in range(2):
                        ps = psp.tile([128, FD], F32, tag="ps")
                        for k in range(2):
                            for n in range(2):
                                nc.tensor.matmul(
                                    ps[:, n * FDH:(n + 1) * FDH],
                                    W1c[:, k * 256 + m * 128: k * 256 + (m + 1) * 128],
                                    ct[k][n][:],
                                    start=(k == 0), stop=(k == 1))
                        nc.vector.tensor_copy(pT[m][:, r0:r0 + FD], ps[:])

            # ---------------- 32 denoising steps (phase-major in waves of 4
            # blocks: deeper PE lookahead within each layer phase)
            BF16 = mybir.dt.bfloat16
            WAVE = 4
            for i in range(T):
                Xc = X[i % 2]
                Xn = X[(i + 1) % 2]
                if i + 1 < T:
                    nc.sync.dma_start(out=Xn[CHUNK:128, :], in_=noise[i + 1])
                for w in range(NB // WAVE):
                    blocks = [w * WAVE + j for j in range(WAVE)]

                    # L1 + cond add + silu
                    h1 = {}
                    for blk in blocks:
                        r0 = blk * FD
                        for m in range(2):
                            ps = psp.tile([128, FD], F32, tag="ps")
                            for n in range(2):
                                nc.tensor.matmul(
                                    ps[:, n * FDH:(n + 1) * FDH],
                                    W1a[:, m * 128:(m + 1) * 128],
                                    Xc[0:CHUNK, r0 + n * FDH: r0 + (n + 1) * FDH],
                                    start=True, stop=True)
                            a1 = wp.tile([128, FD], F32, tag=f"a1_{m}",
                                         name=f"a1_{m}", bufs=3)
                            nc.vector.tensor_tensor(a1[:], ps[:],
                                                    pT[m][:, r0:r0 + FD], op=ADD)
                            h = wp.tile([128, FD], BF16, tag=f"ha_{m}", bufs=6)
                            nc.scalar.activation(h[:], a1[:], SILU,
                                                 bias=b1t[:, i * 2 + m: i * 2 + m + 1])
                            h1[(blk, m)] = h

                    # L2
                    h2 = {}
                    for blk in blocks:
                        for m in range(2):
                            ps = psp.tile([128, FD], F32, tag="ps")
                            for k in range(2):
                                for n in range(2):
                                    nc.tensor.matmul(
                                        ps[:, n * FDH:(n + 1) * FDH],
                                        W2[:, k * 256 + m * 128: k * 256 + (m + 1) * 128],
                                        h1[(blk, k)][:, n * FDH:(n + 1) * FDH],
                                        start=(k == 0), stop=(k == 1))
                            h = wp.tile([128, FD], BF16, tag=f"h2_{m}", bufs=6)
                            nc.scalar.activation(h[:], ps[:], SILU, bias=b2[:, m:m + 1])
                            h2[(blk, m)] = h

                    # L3
                    h3 = {}
                    for blk in blocks:
                        for m in range(2):
                            ps = psp.tile([128, FD], F32, tag="ps")
                            for k in range(2):
                                for n in range(2):
                                    nc.tensor.matmul(
                                        ps[:, n * FDH:(n + 1) * FDH],
                                        W3[:, k * 256 + m * 128: k * 256 + (m + 1) * 128],
                                        h2[(blk, k)][:, n * FDH:(n + 1) * FDH],
                                        start=(k == 0), stop=(k == 1))
                            h = wp.tile([128, FD], BF16, tag=f"ha_{m}", bufs=6)
                            nc.scalar.activation(h[:], ps[:], SILU, bias=b3[:, m:m + 1])
                            h3[(blk, m)] = h

                    # L4 + state/noise add, then copy psum -> next state
                    for blk in blocks:
                        r0 = blk * FD
                        p4 = [ps4p.tile([CHUNK, FDH], F32, tag="ps4",
                                        name="p4", bufs=2) for _ in range(2)]
                        for k in range(2):
                            for n in range(2):
                                nc.tensor.matmul(
                                    p4[n][:],
                                    W4[:, k * T * 64 + i * 64: k * T * 64 + (i + 1) * 64],
                                    h3[(blk, k)][:, n * FDH:(n + 1) * FDH],
                                    start=(k == 0), stop=False)
                        for n in range(2):
                            rs = slice(r0 + n * FDH, r0 + (n + 1) * FDH)
                            nc.tensor.matmul(
                                p4[n][:],
                                L2s[:, i * 64:(i + 1) * 64], Xc[:, rs],
                                start=False, stop=True)
                            nc.vector.tensor_copy(Xn[0:CHUNK, rs], p4[n][:])

            nc.sync.dma_start(out=outT, in_=X[T % 2][0:CHUNK, :])

    nc.finalize()
    return nc


def _prep_inputs(cond, x_init, W1, b1, W2, b2, W3, b3, W4, b4):
    A, Bc, Cc, b1t_d, W4_d, L2_d = _host_tables(W1, b1, W4, b4)
    noiseC = _gen_noise(b4.astype(np.float32), Bc, Cc)

    W1a = np.ascontiguousarray(W1[:CHUNK]).astype(np.float32)
    W1c = np.ascontiguousarray(
        W1[CHUNK + TIME_EMB_DIM:].reshape(2, 128, HIDDEN)).astype(np.float32)
    import ml_dtypes
    W2r = np.ascontiguousarray(W2.reshape(2, 128, HIDDEN)).astype(ml_dtypes.bfloat16)
    W3r = np.ascontiguousarray(W3.reshape(2, 128, HIDDEN)).astype(ml_dtypes.bfloat16)

    def btab(b):
        o = np.empty((128, 2), dtype=np.float32)
        for m in range(2):
            o[:, m] = b[m * 128:(m + 1) * 128]
        return o

    b2_d, b3_d = btab(b2.astype(np.float32)), btab(b3.astype(np.float32))

    condT = np.ascontiguousarray(cond.T).astype(np.float32)      # [256, B]
    xT = np.ascontiguousarray(x_init.T).astype(np.float32)       # [64, B]

    in_maps = []
    for c in range(N_CORES):
        rc = slice(c * R, (c + 1) * R)
        in_maps.append({
            "xT": np.ascontiguousarray(xT[:, rc]),
            "condT": np.ascontiguousarray(condT[:, rc]),
            "noise": np.ascontiguousarray(noiseC[:, :, rc]),
            "W1a": W1a, "W1c": W1c, "W2d": W2r, "W3d": W3r,
            "W4d": W4_d.astype(ml_dtypes.bfloat16), "L2d": L2_d, "b1td": b1t_d,
            "b2d": b2_d, "b3d": b3_d,
        })
    return in_maps


def _install_profile_hook():
    """The agent image's antenv lacks axon_hooks; recreate it so trace=True
    works (used by test.py only)."""
    try:
        import antenv
        if "antenv.axon_hooks" in sys.modules:
            return
        mod = types.ModuleType("antenv.axon_hooks")
        hook = [None]
        mod.set_axon_ntff_profile_hook = lambda h: hook.__setitem__(0, h)
        mod.get_axon_ntff_profile_hook = lambda: hook[0]
        sys.modules["antenv.axon_hooks"] = mod
        antenv.axon_hooks = mod
        from trn_agent_boot.trn_boot import _ntff_profile_via_ctypes
        mod.set_axon_ntff_profile_hook(
            _ntff_profile_via_ctypes("/opt/axon/libaxon_pjrt.so"))
    except Exception:
        pass


_LAST_RESULTS = {}


def kernel(cond, x_init, W1, b1, W2, b2, W3, b3, W4, b4, _trace=False):
    _install_profile_hook()
    from concourse import bass_utils
    bass_utils.upload_artifacts = lambda tmpdir: "local://" + tmpdir
    from concourse.bass_utils import run_bass_kernel_spmd

    _enable_ldw_opt()
    in_maps = _prep_inputs(cond, x_init, W1, b1, W2, b2, W3, b3, W4, b4)
    nc = _build_nc()
    res = run_bass_kernel_spmd(nc, in_maps, list(range(N_CORES)), trace=_trace)
    _LAST_RESULTS["res"] = res

    out = np.empty((B, CHUNK), dtype=np.float32)
    for c in range(N_CORES):
        rc = slice(c * R, (c + 1) * R)
        out[rc] = res.results[c]["outT"].T
    return out.reshape(B, ACTION_HORIZON, ACTION_DIM)


# revision 21
# speedup vs baseline: 1.1345x; 1.0015x over previous
"""Trainium2 Bass kernel for nn_DiffusionHead (32-step DDPM sampling head).

Strategy (pure data parallel, 8 cores, B=65536 -> 8192 rows/core):
  - Transposed activation layout: features on SBUF partitions, batch rows on
    the free dimension. All matmuls in float32r (fp32 storage, ~tf32
    precision, full PE rate).
  - The time-embedding contribution of layer 1 is folded into a per-step
    bias table on the host. The cond contribution of layer 1 (cond @ W1c)
    is step-invariant: computed once on device, kept resident in SBUF.
  - The DDPM update  x_{i+1} = A_i x_i + B_i eps_i + C_i n_i + B_i b4  is
    evaluated inside the layer-4 PSUM accumulation:
      * B_i is folded into a per-step copy of W4 (host table),
      * A_i x_i and the pre-scaled noise are added via one extra matmul
        with a host-built [128,64] block-diagonal operand against a
        combined state tile X = [x^T ; noiseC''^T],
      * C_i n_i + B_i b4 is pre-computed on the host into the noise stream.
    One DVE copy per block moves PSUM into the next state tile.
"""

import math
import os
import sys
import types

import numpy as np

sys.path.insert(0, "/opt/trn_rl_repo")

# ---------------------------------------------------------------- constants
T = 32
BETA_START, BETA_END = 1e-4, 1e-2
ACTION_DIM, ACTION_HORIZON = 4, 16
CHUNK = ACTION_DIM * ACTION_HORIZON          # 64
COND_DIM = 256
TIME_EMB_DIM = 32
HIDDEN = 256
B = 65536
N_CORES = 8
R = B // N_CORES                             # 8192 rows per core
FD = 1024                                    # elementwise block free-dim
NB = R // FD                                 # 8 blocks per step
FDH = 512                                    # matmul moving free-dim (psum bank)


def _host_tables(W1, b1, W4, b4):
    """Per-step coefficient tables, computed in float64 then cast."""
    betas = np.linspace(BETA_START, BETA_END, T, dtype=np.float64)
    alphas = 1.0 - betas
    alpha_bar = np.cumprod(alphas)
    freqs = np.exp(np.linspace(0.0, math.log(1000.0), TIME_EMB_DIM // 2))

    A = np.empty(T)
    Bc = np.empty(T)
    Cc = np.empty(T)
    for i in range(T):
        t = T - 1 - i
        ab, al, be = alpha_bar[t], alphas[t], betas[t]
        if t > 0:
            A[i] = math.sqrt(al / ab)
            Bc[i] = -math.sqrt(al * (1.0 - ab) / ab)
            Cc[i] = math.sqrt(be)
        else:
            A[i] = 1.0 / math.sqrt(ab)
            Bc[i] = -math.sqrt((1.0 - ab) / ab)
            Cc[i] = 0.0

    # layer-1 bias per step: b1 + t_emb(t) @ W1[64:96]
    W1 = W1.astype(np.float64)
    b1t = np.empty((T, HIDDEN))
    for i in range(T):
        t = T - 1 - i
        args = t * freqs
        temb = np.concatenate([np.sin(args), np.cos(args)])
        b1t[i] = b1.astype(np.float64) + temb @ W1[CHUNK:CHUNK + TIME_EMB_DIM]

    # b1t table for SBUF: [128, T*2], [p, i*2+m] = b1t[i][m*128+p]
    b1t_d = np.empty((128, T * 2), dtype=np.float32)
    for i in range(T):
        for m in range(2):
            b1t_d[:, i * 2 + m] = b1t[i, m * 128:(m + 1) * 128]

    # W4 table: [2, 128, T*64], [k, p, i*64+m] = B_i * W4[k*128+p, m]
    W4_d = np.empty((2, 128, T * 64), dtype=np.float32)
    for i in range(T):
        for k in range(2):
            W4_d[k][:, i * 64:(i + 1) * 64] = Bc[i] * W4[k * 128:(k + 1) * 128, :]

    # state/noise matmul operand: [128, T*64]
    # rows 0..63:  A_i * I64   (x part);  rows 64..127: I64 (noise part)
    L2_d = np.zeros((128, T * 64), dtype=np.float32)
    eye = np.eye(CHUNK, dtype=np.float32)
    for i in range(T):
        L2_d[0:64, i * 64:(i + 1) * 64] = A[i] * eye
        L2_d[64:128, i * 64:(i + 1) * 64] = eye

    return A, Bc, Cc, b1t_d, W4_d, L2_d


def _gen_noise(b4, Bc, Cc):
    """Reference-matching threefry noise, pre-scaled:
    noiseC[i] = C_i * n_i^T + B_i * b4[:, None]   shape [T, 64, B]"""
    import jax
    import jax.numpy as jnp

    out = np.empty((T, CHUNK, B), dtype=np.float32)
    keys = jax.random.split(jax.random.key(42), T)
    for i in range(T):
        if Cc[i] != 0.0:
            n = np.asarray(jax.random.normal(keys[i], (B, CHUNK), jnp.float32))
            out[i] = (np.float32(Cc[i]) * n.T
                      + np.float32(Bc[i]) * b4[:, None])
        else:
            out[i] = (np.float32(Bc[i]) * b4[:, None]) * np.ones((1, B), np.float32)
    return out



def _enable_ldw_opt():
    """Flip walrus --enable-ldw-opt to true (dedups repeated weight loads;
    halves PE LDWEIGHTS time for our consecutive same-weight matmuls)."""
    return  # disabled: conflicts with explicit bf16 InstLdweights
    from concourse import bass_utils as _bu
    if getattr(_bu, "_ldw_opt_patched", False):
        return
    _orig = _bu.run_command

    def _run(cmd, *a, **kw):
        if isinstance(cmd, list) and "--enable-ldw-opt=false" in cmd:
            cmd = ["--enable-ldw-opt=true" if c == "--enable-ldw-opt=false" else c
                   for c in cmd]
        return _orig(cmd, *a, **kw)

    _bu.run_command = _run
    _bu._ldw_opt_patched = True

def _build_nc():
    import concourse.bacc as bacc
    import concourse.mybir as mybir
    from concourse.tile import TileContext

    F32R = mybir.dt.float32r
    F32 = mybir.dt.float32
    SILU = mybir.ActivationFunctionType.Silu
    ADD = mybir.AluOpType.add

    nc = bacc.Bacc("TRN2", target_bir_lowering=False, debug=False,
                   num_devices=N_CORES)

    xT = nc.dram_tensor("xT", [CHUNK, R], F32R, kind="ExternalInput").ap()
    condT = nc.dram_tensor("condT", [COND_DIM, R], F32R, kind="ExternalInput").ap()
    noise = nc.dram_tensor("noise", [T, CHUNK, R], F32R, kind="ExternalInput").ap()
    W1a_d = nc.dram_tensor("W1a", [CHUNK, HIDDEN], F32R, kind="ExternalInput").ap()
    W1c_d = nc.dram_tensor("W1c", [2, 128, HIDDEN], F32R, kind="ExternalInput").ap()
    BF16D = mybir.dt.bfloat16
    W2_d = nc.dram_tensor("W2d", [2, 128, HIDDEN], BF16D, kind="ExternalInput").ap()
    W3_d = nc.dram_tensor("W3d", [2, 128, HIDDEN], BF16D, kind="ExternalInput").ap()
    W4_d = nc.dram_tensor("W4d", [2, 128, T * 64], BF16D, kind="ExternalInput").ap()
    L2_d = nc.dram_tensor("L2d", [128, T * 64], F32R, kind="ExternalInput").ap()
    b1t_d = nc.dram_tensor("b1td", [128, T * 2], F32, kind="ExternalInput").ap()
    b2_d = nc.dram_tensor("b2d", [128, 2], F32, kind="ExternalInput").ap()
    b3_d = nc.dram_tensor("b3d", [128, 2], F32, kind="ExternalInput").ap()
    outT = nc.dram_tensor("outT", [CHUNK, R], F32R, kind="ExternalOutput").ap()

    with TileContext(nc) as tc:
        with tc.tile_pool(name="persist", bufs=1) as pp, \
             tc.tile_pool(name="work", bufs=2) as wp, \
             tc.tile_pool(name="psum", bufs=3, space="PSUM") as psp, \
             tc.tile_pool(name="psum4", bufs=1, space="PSUM") as ps4p:

            # ---------------- persistent tiles
            X = [pp.tile([128, R], F32R, tag="x0", name="x0"),
                 pp.tile([128, R], F32R, tag="x1", name="x1")]
            BF16 = mybir.dt.bfloat16
            pT = [pp.tile([128, R], BF16, tag="pt0", name="pt0"),
                  pp.tile([128, R], BF16, tag="pt1", name="pt1")]
            W1a = pp.tile([CHUNK, HIDDEN], F32R, tag="w1a")
            W2 = pp.tile([128, 2 * HIDDEN], mybir.dt.bfloat16, tag="w2")
            W3 = pp.tile([128, 2 * HIDDEN], mybir.dt.bfloat16, tag="w3")
            W4 = pp.tile([128, 2 * T * 64], mybir.dt.bfloat16, tag="w4")
            L2s = pp.tile([128, T * 64], F32R, tag="l2s")
            b1t = pp.tile([128, T * 2], F32, tag="b1t")
            b2 = pp.tile([128, 2], F32, tag="b2")
            b3 = pp.tile([128, 2], F32, tag="b3")

            nc.sync.dma_start(out=W1a[:], in_=W1a_d)
            for k in range(2):
                nc.sync.dma_start(out=W2[:, k * 256:(k + 1) * 256], in_=W2_d[k])
                nc.sync.dma_start(out=W3[:, k * 256:(k + 1) * 256], in_=W3_d[k])
                nc.sync.dma_start(out=W4[:, k * T * 64:(k + 1) * T * 64], in_=W4_d[k])
            nc.sync.dma_start(out=L2s[:], in_=L2_d)
            nc.sync.dma_start(out=b1t[:], in_=b1t_d)
            nc.sync.dma_start(out=b2[:], in_=b2_d)
            nc.sync.dma_start(out=b3[:], in_=b3_d)
            nc.sync.dma_start(out=X[0][0:CHUNK, :], in_=xT)
            nc.sync.dma_start(out=X[0][CHUNK:128, :], in_=noise[0])

            # ---------------- one-time cond projection: pT = (cond @ W1c)^T
            with tc.tile_pool(name="init", bufs=2) as ip:
                W1c = ip.tile([128, 2 * HIDDEN], F32R, tag="w1c", bufs=1)
                for k in range(2):
                    nc.sync.dma_start(out=W1c[:, k * 256:(k + 1) * 256], in_=W1c_d[k])
                for blk in range(NB):
                    r0 = blk * FD
                    ct = [[None, None], [None, None]]
                    for k in range(2):
                        for n in range(2):
                            c = ip.tile([128, FDH], F32R, tag=f"c{k}{n}",
                                        name=f"ct{k}{n}", bufs=1)
                            nc.sync.dma_start(
                                out=c[:],
                                in_=condT[k * 128:(k + 1) * 128,
                                          r0 + n * FDH: r0 + (n + 1) * FDH])
                            ct[k][n] = c
                    for m in range(2):
                        ps = psp.tile([128, FD], F32, tag="ps")
                        for k in range(2):
                            for n in range(2):
                                nc.tensor.matmul(
                                    ps[:, n * FDH:(n + 1) * FDH],
                                    W1c[:, k * 256 + m * 128: k * 256 + (m + 1) * 128],
                                    ct[k][n][:],
                                    start=(k == 0), stop=(k == 1))
                        nc.vector.tensor_copy(pT[m][:, r0:r0 + FD], ps[:])

            # ---------------- 32 denoising steps (phase-major in waves of 4
            # blocks: deeper PE lookahead within each layer phase)
            BF16 = mybir.dt.bfloat16
            WAVE = 4
            for i in range(T):
                Xc = X[i % 2]
                Xn = X[(i + 1) % 2]
                if i + 1 < T:
                    for q in range(4):
                        qs = slice(q * (R // 4), (q + 1) * (R // 4))
                        nc.sync.dma_start(out=Xn[CHUNK:128, qs],
                                          in_=noise[i + 1][:, qs])
                for w in range(NB // WAVE):
                    blocks = [w * WAVE + j for j in range(WAVE)]

                    # L1 + cond add + silu
                    h1 = {}
                    for blk in blocks:
                        r0 = blk * FD
                        for m in range(2):
                            ps = psp.tile([128, FD], F32, tag="ps")
                            for n in range(2):
                                nc.tensor.matmul(
                                    ps[:, n * FDH:(n + 1) * FDH],
                                    W1a[:, m * 128:(m + 1) * 128],
                                    Xc[0:CHUNK, r0 + n * FDH: r0 + (n + 1) * FDH],
                                    start=True, stop=True)
                            a1 = wp.tile([128, FD], F32, tag=f"a1_{m}",
                                         name=f"a1_{m}", bufs=3)
                            nc.vector.tensor_tensor(a1[:], ps[:],
                                                    pT[m][:, r0:r0 + FD], op=ADD)
                            h = wp.tile([128, FD], BF16, tag=f"ha_{m}", bufs=6)
                            nc.scalar.activation(h[:], a1[:], SILU,
                                                 bias=b1t[:, i * 2 + m: i * 2 + m + 1])
                            h1[(blk, m)] = h

                    # L2
                    h2 = {}
                    for blk in blocks:
                        for m in range(2):
                            ps = psp.tile([128, FD], F32, tag="ps")
                            for k in range(2):
                                for n in range(2):
                                    nc.tensor.matmul(
                                        ps[:, n * FDH:(n + 1) * FDH],
                                        W2[:, k * 256 + m * 128: k * 256 + (m + 1) * 128],
                                        h1[(blk, k)][:, n * FDH:(n + 1) * FDH],
                                        start=(k == 0), stop=(k == 1))
                            h = wp.tile([128, FD], BF16, tag=f"h2_{m}", bufs=6)
                            nc.scalar.activation(h[:], ps[:], SILU, bias=b2[:, m:m + 1])
                            h2[(blk, m)] = h

                    # L3
                    h3 = {}
                    for blk in blocks:
                        for m in range(2):
                            ps = psp.tile([128, FD], F32, tag="ps")
                            for k in range(2):
                                for n in range(2):
                                    nc.tensor.matmul(
                                        ps[:, n * FDH:(n + 1) * FDH],
                                        W3[:, k * 256 + m * 128: k * 256 + (m + 1) * 128],
                                        h2[(blk, k)][:, n * FDH:(n + 1) * FDH],
                                        start=(k == 0), stop=(k == 1))
                            h = wp.tile([128, FD], BF16, tag=f"ha_{m}", bufs=6)
                            nc.scalar.activation(h[:], ps[:], SILU, bias=b3[:, m:m + 1])
                            h3[(blk, m)] = h

                    # L4 + state/noise add, then copy psum -> next state
                    for blk in blocks:
                        r0 = blk * FD
                        p4 = [ps4p.tile([CHUNK, FDH], F32, tag="ps4",
                                        name="p4", bufs=2) for _ in range(2)]
                        for k in range(2):
                            for n in range(2):
                                nc.tensor.matmul(
                                    p4[n][:],
                                    W4[:, k * T * 64 + i * 64: k * T * 64 + (i + 1) * 64],
                                    h3[(blk, k)][:, n * FDH:(n + 1) * FDH],
                                    start=(k == 0), stop=False)
                        for n in range(2):
                            rs = slice(r0 + n * FDH, r0 + (n + 1) * FDH)
                            nc.tensor.matmul(
                                p4[n][:],
                                L2s[:, i * 64:(i + 1) * 64], Xc[:, rs],
                                start=False, stop=True)
                            nc.vector.tensor_copy(Xn[0:CHUNK, rs], p4[n][:])

            nc.sync.dma_start(out=outT, in_=X[T % 2][0:CHUNK, :])

    nc.finalize()
    return nc


def _prep_inputs(cond, x_init, W1, b1, W2, b2, W3, b3, W4, b4):
    A, Bc, Cc, b1t_d, W4_d, L2_d = _host_tables(W1, b1, W4, b4)
    noiseC = _gen_noise(b4.astype(np.float32), Bc, Cc)

    W1a = np.ascontiguousarray(W1[:CHUNK]).astype(np.float32)
    W1c = np.ascontiguousarray(
        W1[CHUNK + TIME_EMB_DIM:].reshape(2, 128, HIDDEN)).astype(np.float32)
    import ml_dtypes
    W2r = np.ascontiguousarray(W2.reshape(2, 128, HIDDEN)).astype(ml_dtypes.bfloat16)
    W3r = np.ascontiguousarray(W3.reshape(2, 128, HIDDEN)).astype(ml_dtypes.bfloat16)

    def btab(b):
        o = np.empty((128, 2), dtype=np.float32)
        for m in range(2):
            o[:, m] = b[m * 128:(m + 1) * 128]
        return o

    b2_d, b3_d = btab(b2.astype(np.float32)), btab(b3.astype(np.float32))

    condT = np.ascontiguousarray(cond.T).astype(np.float32)      # [256, B]
    xT = np.ascontiguousarray(x_init.T).astype(np.float32)       # [64, B]

    in_maps = []
    for c in range(N_CORES):
        rc = slice(c * R, (c + 1) * R)
        in_maps.append({
            "xT": np.ascontiguousarray(xT[:, rc]),
            "condT": np.ascontiguousarray(condT[:, rc]),
            "noise": np.ascontiguousarray(noiseC[:, :, rc]),
            "W1a": W1a, "W1c": W1c, "W2d": W2r, "W3d": W3r,
            "W4d": W4_d.astype(ml_dtypes.bfloat16), "L2d": L2_d, "b1td": b1t_d,
            "b2d": b2_d, "b3d": b3_d,
        })
    return in_maps


def _install_profile_hook():
    """The agent image's antenv lacks axon_hooks; recreate it so trace=True
    works (used by test.py only)."""
    try:
        import antenv
        if "antenv.axon_hooks" in sys.modules:
            return
        mod = types.ModuleType("antenv.axon_hooks")
        hook = [None]
        mod.set_axon_ntff_profile_hook = lambda h: hook.__setitem__(0, h)
        mod.get_axon_ntff_profile_hook = lambda: hook[0]
        sys.modules["antenv.axon_hooks"] = mod
        antenv.axon_hooks = mod
        from trn_agent_boot.trn_boot import _ntff_profile_via_ctypes
        mod.set_axon_ntff_profile_hook(
            _ntff_profile_via_ctypes("/opt/axon/libaxon_pjrt.so"))
    except Exception:
        pass


_LAST_RESULTS = {}


def kernel(cond, x_init, W1, b1, W2, b2, W3, b3, W4, b4, _trace=False):
    _install_profile_hook()
    from concourse import bass_utils
    bass_utils.upload_artifacts = lambda tmpdir: "local://" + tmpdir
    from concourse.bass_utils import run_bass_kernel_spmd

    _enable_ldw_opt()
    in_maps = _prep_inputs(cond, x_init, W1, b1, W2, b2, W3, b3, W4, b4)
    nc = _build_nc()
    res = run_bass_kernel_spmd(nc, in_maps, list(range(N_CORES)), trace=_trace)
    _LAST_RESULTS["res"] = res

    out = np.empty((B, CHUNK), dtype=np.float32)
    for c in range(N_CORES):
        rc = slice(c * R, (c + 1) * R)
        out[rc] = res.results[c]["outT"].T
    return out.reshape(B, ACTION_HORIZON, ACTION_DIM)
